# revision 34
# baseline (speedup 1.0000x reference)
"""DiffAugment (flip / brightness / contrast / translation / cutout) on
Trainium2, data-parallel over 8 NeuronCores (8 samples per core).

Every per-sample augmentation decision is folded on the host into a small set
of per-sample parameters; the device runs one uniform SPMD Bass/Tile program
whose only data-dependent behavior is two runtime register values per sample.

Host builds xpad2 [S, 3, 288, 544]:
  rows: 16 zero-margin + 256 payload + 16 zero-margin  (translation row shift
        becomes a dynamic row offset; out-of-range rows are zeroed by a mask)
  cols: [0:256] raw x columns; [256:544] the 255-periodic translation ring
        B[k] = x[(k-16) % 255], with ring cells B[16]/B[271] patched to
        x[.,255] for flipped+translated samples (the single column where
        flip-then-translate and translate-then-flip disagree).

Device, per sample (s = 3*h + c subtiles, rows on partitions):
  T [128, 6, 256]  <- one dynamic linear offset (lin_off = row*544 + col)
                      picks the entire translated window (2 DMAs, row halves)
  O [128, 6, 256, 2]:
    slot0     = scale*T + bias              (ACT, fully static)
    slot ds(z)= scale*reverse(T) + bias     (DVE tensor_scalar, static
               reversed input; z = 0 if flipped else 1, so a flipped sample's
               reversed image overwrites slot0 and slot1 is a scratch bin)
  M[h] = cm_rep * a[h] + rv[h]   (rank-1 mask: cutout rectangle AND
                                  out-of-range translation rows)
  Of[:, 3h+c, :] = O[:, 3h+c, :, 0] * M[h];  store Of -> y[b]
"""
import sys
import numpy as np

for _p in ("/opt/trn_rl_repo",):
    if _p not in sys.path:
        sys.path.insert(0, _p)

import concourse.bass as bass
import concourse.mybir as mybir
from concourse.ap import AP
from concourse.tile import TileContext
from concourse.vector_clock import ScopedClock, VectorClock
from concourse.bass_utils import run_bass_kernel_spmd


class _SplitDrainTileContext(TileContext):
    """TileContext whose kernel-tail drain pre-absorbs its semaphore waits
    into one NOP per outstanding semaphore (this walrus flow packs at most
    one sync wait into any TPB instruction)."""

    def _drain_and_barrier(self, tick_clock, wait_clock):
        full = tick_clock.global_clock
        vals = [full[i] for i in range(27)]
        nz = [i for i, v in enumerate(vals) if v > 0]
        for i in nz:
            cv = [vals[j] if j == i else 0 for j in range(27)]
            nop = self.nc.sync.nop(nofuse=True)
            wait_clock.add_sem_waits(nop.ins,
                                     ScopedClock({None: VectorClock(cv)}))
        # the NOPs above already waited on every outstanding semaphore, so
        # the drain itself carries no sem waits (original code attaches all
        # of them to this one instruction, which overflows its wait slots)
        self.nc.sync.drain()
        self.nc.all_engine_barrier()
        assert self.sems is not None
        popped = self.nc._tile_sem_poison_stack.pop()
        assert popped is self._sem_poison
        self.nc.clear_and_free_semaphores(list(self.sems.allocated().values()))
        self.nc.all_engine_barrier()

N_CORES = 8
S = 8                      # samples per core
B, C, H, W = 64, 3, 256, 256
PAD_TOP = 16
FLATR = 800                # flat rows: 16 + 3*256 + 16
TW = 544                   # xpad2 col width: raw 256 + ring 288
NI = 4                     # ints per sample
F32 = np.float32

_IDENT = mybir.ActivationFunctionType.Identity
_ET = mybir.EngineType
_MULT = mybir.AluOpType.mult
_ADD = mybir.AluOpType.add


# --------------------------------------------------------------------------
# Host-side parameter derivation
# --------------------------------------------------------------------------
def _derive_params(x, p, flip_u, bright_n, bright_u, contrast_n, contrast_u,
                   trans_h, trans_w, trans_u, cut_ox, cut_oy, cut_u):
    x = np.asarray(x, np.float32)
    p = F32(np.asarray(p).reshape(()))
    flip_u = np.asarray(flip_u, np.float32).reshape(B)
    bright_n = np.asarray(bright_n, np.float32).reshape(B)
    bright_u = np.asarray(bright_u, np.float32).reshape(B)
    contrast_n = np.asarray(contrast_n, np.float32).reshape(B)
    contrast_u = np.asarray(contrast_u, np.float32).reshape(B)
    trans_h = np.asarray(trans_h).reshape(B).astype(np.int64)
    trans_w = np.asarray(trans_w).reshape(B).astype(np.int64)
    trans_u = np.asarray(trans_u, np.float32).reshape(B)
    cut_ox = np.asarray(cut_ox).reshape(B).astype(np.int64)
    cut_oy = np.asarray(cut_oy).reshape(B).astype(np.int64)
    cut_u = np.asarray(cut_u, np.float32).reshape(B)

    flip = flip_u < F32(0.5) * p
    trans = trans_u < p
    cut = cut_u < p

    th = np.where(trans, trans_h, 0)
    tw = np.where(trans, trans_w, 0)

    scale = np.where(contrast_u < p, np.exp2(contrast_n * F32(0.5)),
                     F32(1.0)).astype(F32)
    add = np.where(bright_u < p, bright_n * F32(0.2), F32(0.0)).astype(F32)
    bias = (add * scale).astype(F32)

    xflat = x.reshape(B, C * H, W)
    xpad3 = np.zeros((B, FLATR, TW), np.float32)
    xpad3[:, PAD_TOP:PAD_TOP + C * H, 0:256] = xflat
    ring_idx = np.concatenate([np.arange(239, 255),
                               np.arange(0, 255),
                               np.arange(0, 17)])
    xpad3[:, PAD_TOP:PAD_TOP + C * H, 256:544] = xflat[:, :, ring_idx]
    patched = flip & trans
    xpad3[patched, PAD_TOP:PAD_TOP + C * H, 256 + 16] = xflat[patched, :, 255]
    xpad3[patched, PAD_TOP:PAD_TOP + C * H, 256 + 271] = xflat[patched, :, 255]

    col_off = np.where(trans,
                       np.where(flip, 256 + 16 - tw, 256 + 16 + tw),
                       0).astype(np.int64)
    lin_off = ((PAD_TOP + th) * TW + col_off).astype(np.int32)
    z_slot = np.where(flip, 0, 1).astype(np.int32)

    i_idx = np.arange(H)
    rowvalid = ((i_idx[None, :] + th[:, None] >= 0)
                & (i_idx[None, :] + th[:, None] <= H - 1)).astype(F32)
    r0 = np.clip(cut_ox - 64, 0, H - 1)
    r1 = np.clip(cut_ox + 63, 0, H - 1)
    c0 = np.clip(cut_oy - 64, 0, W - 1)
    c1 = np.clip(cut_oy + 63, 0, W - 1)
    rm = ((i_idx[None, :] >= r0[:, None]) & (i_idx[None, :] <= r1[:, None])
          & cut[:, None]).astype(F32)
    cm = ((i_idx[None, :] >= c0[:, None]) & (i_idx[None, :] <= c1[:, None])
          & cut[:, None]).astype(F32)

    return {
        "xpad3": xpad3,
        "scl": scale,
        "bia": bias,
        "av": (-(rowvalid * rm)).astype(F32).reshape(B, 2, 128),
        "rv": rowvalid.reshape(B, 2, 128).copy(),
        "cm": cm,
        "lin": lin_off,
        "z": z_slot,
    }


# --------------------------------------------------------------------------
def _build_nc():
    # Wait-count discipline (this walrus flow allows only ONE sync wait per
    # TPB/DMA instruction):
    #  - all per-sample scalars/ints/cut-masks ship in ONE packed tensor
    #    (pars; ints bitcast into f32 columns), one DMA, one absorber copy
    #  - one 3D DMA per sample per direction (channels flattened into the
    #    row axis with constant 128-row subtile stride, order s = 2c+h)
    #  - all compute on DVE, where same-engine ordering needs no semaphores
    #  - pool bufs=S so there are no slot-reuse waits at all
    nc = bass.Bass(trn_type="TRN2")
    f32, i32 = mybir.dt.float32, mybir.dt.int32
    xpad3 = nc.dram_tensor("xpad3", [S, FLATR, TW], f32, kind="ExternalInput")
    pars = nc.dram_tensor("pars", [128, 8 * S + 256 * S], f32,
                          kind="ExternalInput")
    y = nc.dram_tensor("y", [S, C, H, W], f32, kind="ExternalOutput")

    with _SplitDrainTileContext(nc) as tc:
        with tc.tile_pool(name="const", bufs=1) as cpool, \
             tc.tile_pool(name="work", bufs=S) as wpool:
            parsT = cpool.tile([128, 8 * S + 256 * S], f32)
            scr = cpool.tile([128, 4], f32)
            nc.sync.dma_start(parsT, pars[:, :])
            # absorber: soak up the parsT DMA wait on DVE once, so the
            # 1-wait-budget TensorScalarPtr ops below never see it
            nc.vector.tensor_copy(scr[:, 0:1], parsT[:, 0:1])

            for b in range(S):
                T = wpool.tile([128, 6, 256], f32, tag="T")
                # OO[:,0] = affine image; OO[:,1] = masked output (also the
                # scratch bin for the reversed select of unflipped samples)
                OO = wpool.tile([128, 2, 6, 256], f32, tag="OO")
                Mh = wpool.tile([128, 2, 256], f32, tag="Mh")

                def iv(col, lo, hi, eng):
                    return nc.values_load(
                        parsT[0:1, col:col + 1].bitcast(i32),
                        engines=[eng], min_val=lo, max_val=hi,
                        skip_runtime_bounds_check=True)

                # ---- load: one 3D DMA, dynamic linear offset ----
                lin = iv(6 * S + 2 * b, 0, 32 * TW + 288, _ET.SP)
                src = AP(xpad3, b * (FLATR * TW) + lin,
                         [[TW, 128], [128 * TW, 6], [1, 256]])
                nc.sync.dma_start(T[:, :, :], src)

                # ---- rank-1 mask build (cutout rect + invalid rows) ----
                for h in (0, 1):
                    col = 2 * S + 2 * b + h
                    nc.vector.tensor_scalar(
                        Mh[:, h], parsT[:, 8 * S + 256 * b:8 * S + 256 * b + 256],
                        parsT[:, col:col + 1],
                        parsT[:, 2 * S + col:2 * S + col + 1], _MULT, _ADD)

                # ---- affine selects (fwd -> slot0; reversed -> slot z,
                # which is slot0 for flipped samples, else the scratch
                # slot1 that the mask-apply overwrites in order) ----
                sc = parsT[:, b:b + 1]
                bi = parsT[:, S + b:S + b + 1]
                nc.vector.tensor_scalar(
                    OO[:, 0], T, sc, bi, _MULT, _ADD)
                z = iv(6 * S + 2 * b + 1, 0, 1, _ET.DVE)
                nc.vector.tensor_scalar(
                    OO[:, bass.ds(z, 1)],
                    T[:, :, ::-1].unsqueeze(1), sc, bi, _MULT, _ADD)

                # ---- mask apply (halves differ in per-partition scalars) ----
                for h in (0, 1):
                    nc.vector.tensor_mul(
                        OO[:, 1, h::2, :],
                        OO[:, 0, h::2, :],
                        Mh[:, h:h + 1, :].broadcast_to((128, 3, 256)))

                # ---- store: one 3D DMA into flat output rows ----
                dst = AP(y, b * (C * H * W),
                         [[256, 128], [128 * 256, 6], [1, 256]])
                nc.gpsimd.dma_start(dst, OO[:, 1])
    return nc


_NC = None


def _get_nc():
    global _NC
    if _NC is None:
        _NC = _build_nc()
    return _NC


def _shard(params, k):
    lo, hi = k * S, (k + 1) * S
    pars = np.zeros((128, 8 * S + 256 * S), np.float32)
    pars[:, 0:S] = params["scl"][lo:hi][None, :]
    pars[:, S:2 * S] = params["bia"][lo:hi][None, :]
    pars[:, 2 * S:4 * S] = params["av"][lo:hi].reshape(S * 2, 128).T
    pars[:, 4 * S:6 * S] = params["rv"][lo:hi].reshape(S * 2, 128).T
    ints = np.stack([params["lin"][lo:hi], params["z"][lo:hi]],
                    axis=1).reshape(2 * S).astype(np.int32)
    pars[:, 6 * S:8 * S] = ints.view(np.float32)[None, :]
    pars[:, 8 * S:] = np.broadcast_to(
        params["cm"][lo:hi].reshape(S * 256)[None, :], (128, S * 256))
    return {
        "xpad3": np.ascontiguousarray(params["xpad3"][lo:hi]),
        "pars": pars,
    }


def kernel(**inputs) -> np.ndarray:
    params = _derive_params(**{k: np.asarray(v) for k, v in inputs.items()})
    in_maps = [_shard(params, k) for k in range(N_CORES)]
    nc = _get_nc()
    res = run_bass_kernel_spmd(nc, in_maps, core_ids=list(range(N_CORES)))
    out = np.concatenate([np.asarray(r["y"], np.float32)
                          for r in res.results], axis=0)
    return np.ascontiguousarray(out)


if __name__ == "__main__":
    rng = np.random.default_rng(0)
    demo = {
        "x": rng.standard_normal((B, C, H, W)).astype(np.float32),
        "p": np.full((1,), 0.6, np.float32),
        "flip_u": rng.random(B).astype(np.float32),
        "bright_n": rng.standard_normal((B, 1, 1, 1)).astype(np.float32),
        "bright_u": rng.random((B, 1, 1, 1)).astype(np.float32),
        "contrast_n": rng.standard_normal((B, 1, 1, 1)).astype(np.float32),
        "contrast_u": rng.random((B, 1, 1, 1)).astype(np.float32),
        "trans_h": rng.integers(-16, 17, (B, 1, 1)).astype(np.int32),
        "trans_w": rng.integers(-16, 17, (B, 1, 1)).astype(np.int32),
        "trans_u": rng.random(B).astype(np.float32),
        "cut_ox": rng.integers(0, 257, (B, 1, 1)).astype(np.int32),
        "cut_oy": rng.integers(0, 257, (B, 1, 1)).astype(np.int32),
        "cut_u": rng.random(B).astype(np.float32),
    }
    out = kernel(**demo)
    print("kernel output:", out.shape, out.dtype)


# revision 35
# speedup vs baseline: 1.0337x; 1.0337x over previous
"""DiffAugment (flip / brightness / contrast / translation / cutout) on
Trainium2, data-parallel over 8 NeuronCores (8 samples per core).

Every per-sample augmentation decision is folded on the host into a small set
of per-sample parameters; the device runs one uniform SPMD Bass/Tile program
whose only data-dependent behavior is two runtime register values per sample.

Host builds xpad2 [S, 3, 288, 544]:
  rows: 16 zero-margin + 256 payload + 16 zero-margin  (translation row shift
        becomes a dynamic row offset; out-of-range rows are zeroed by a mask)
  cols: [0:256] raw x columns; [256:544] the 255-periodic translation ring
        B[k] = x[(k-16) % 255], with ring cells B[16]/B[271] patched to
        x[.,255] for flipped+translated samples (the single column where
        flip-then-translate and translate-then-flip disagree).

Device, per sample (s = 3*h + c subtiles, rows on partitions):
  T [128, 6, 256]  <- one dynamic linear offset (lin_off = row*544 + col)
                      picks the entire translated window (2 DMAs, row halves)
  O [128, 6, 256, 2]:
    slot0     = scale*T + bias              (ACT, fully static)
    slot ds(z)= scale*reverse(T) + bias     (DVE tensor_scalar, static
               reversed input; z = 0 if flipped else 1, so a flipped sample's
               reversed image overwrites slot0 and slot1 is a scratch bin)
  M[h] = cm_rep * a[h] + rv[h]   (rank-1 mask: cutout rectangle AND
                                  out-of-range translation rows)
  Of[:, 3h+c, :] = O[:, 3h+c, :, 0] * M[h];  store Of -> y[b]
"""
import sys
import numpy as np

for _p in ("/opt/trn_rl_repo",):
    if _p not in sys.path:
        sys.path.insert(0, _p)

import concourse.bass as bass
import concourse.mybir as mybir
from concourse.ap import AP
from concourse.tile import TileContext
from concourse.vector_clock import ScopedClock, VectorClock
from concourse.bass_utils import run_bass_kernel_spmd


class _SplitDrainTileContext(TileContext):
    """TileContext whose kernel-tail drain pre-absorbs its semaphore waits
    into one NOP per outstanding semaphore (this walrus flow packs at most
    one sync wait into any TPB instruction)."""

    def _drain_and_barrier(self, tick_clock, wait_clock):
        full = tick_clock.global_clock
        vals = [full[i] for i in range(27)]
        nz = [i for i, v in enumerate(vals) if v > 0]
        for i in nz:
            cv = [vals[j] if j == i else 0 for j in range(27)]
            nop = self.nc.sync.nop(nofuse=True)
            wait_clock.add_sem_waits(nop.ins,
                                     ScopedClock({None: VectorClock(cv)}))
        # the NOPs above already waited on every outstanding semaphore, so
        # the drain itself carries no sem waits (original code attaches all
        # of them to this one instruction, which overflows its wait slots)
        self.nc.sync.drain()
        self.nc.all_engine_barrier()
        assert self.sems is not None
        popped = self.nc._tile_sem_poison_stack.pop()
        assert popped is self._sem_poison
        self.nc.clear_and_free_semaphores(list(self.sems.allocated().values()))
        self.nc.all_engine_barrier()

N_CORES = 8
S = 8                      # samples per core
B, C, H, W = 64, 3, 256, 256
PAD_TOP = 16
FLATR = 800                # flat rows: 16 + 3*256 + 16
TW = 544                   # xpad2 col width: raw 256 + ring 288
NI = 4                     # ints per sample
F32 = np.float32

_IDENT = mybir.ActivationFunctionType.Identity
_ET = mybir.EngineType
_MULT = mybir.AluOpType.mult
_ADD = mybir.AluOpType.add


# --------------------------------------------------------------------------
# Host-side parameter derivation
# --------------------------------------------------------------------------
def _derive_params(x, p, flip_u, bright_n, bright_u, contrast_n, contrast_u,
                   trans_h, trans_w, trans_u, cut_ox, cut_oy, cut_u):
    x = np.asarray(x, np.float32)
    p = F32(np.asarray(p).reshape(()))
    flip_u = np.asarray(flip_u, np.float32).reshape(B)
    bright_n = np.asarray(bright_n, np.float32).reshape(B)
    bright_u = np.asarray(bright_u, np.float32).reshape(B)
    contrast_n = np.asarray(contrast_n, np.float32).reshape(B)
    contrast_u = np.asarray(contrast_u, np.float32).reshape(B)
    trans_h = np.asarray(trans_h).reshape(B).astype(np.int64)
    trans_w = np.asarray(trans_w).reshape(B).astype(np.int64)
    trans_u = np.asarray(trans_u, np.float32).reshape(B)
    cut_ox = np.asarray(cut_ox).reshape(B).astype(np.int64)
    cut_oy = np.asarray(cut_oy).reshape(B).astype(np.int64)
    cut_u = np.asarray(cut_u, np.float32).reshape(B)

    flip = flip_u < F32(0.5) * p
    trans = trans_u < p
    cut = cut_u < p

    th = np.where(trans, trans_h, 0)
    tw = np.where(trans, trans_w, 0)

    scale = np.where(contrast_u < p, np.exp2(contrast_n * F32(0.5)),
                     F32(1.0)).astype(F32)
    add = np.where(bright_u < p, bright_n * F32(0.2), F32(0.0)).astype(F32)
    bias = (add * scale).astype(F32)

    xflat = x.reshape(B, C * H, W)
    xpad3 = np.zeros((B, FLATR, TW), np.float32)
    xpad3[:, PAD_TOP:PAD_TOP + C * H, 0:256] = xflat
    ring_idx = np.concatenate([np.arange(239, 255),
                               np.arange(0, 255),
                               np.arange(0, 17)])
    xpad3[:, PAD_TOP:PAD_TOP + C * H, 256:544] = xflat[:, :, ring_idx]
    patched = flip & trans
    xpad3[patched, PAD_TOP:PAD_TOP + C * H, 256 + 16] = xflat[patched, :, 255]
    xpad3[patched, PAD_TOP:PAD_TOP + C * H, 256 + 271] = xflat[patched, :, 255]

    col_off = np.where(trans,
                       np.where(flip, 256 + 16 - tw, 256 + 16 + tw),
                       0).astype(np.int64)
    lin_off = ((PAD_TOP + th) * TW + col_off).astype(np.int32)
    z_slot = np.where(flip, 0, 1).astype(np.int32)

    i_idx = np.arange(H)
    rowvalid = ((i_idx[None, :] + th[:, None] >= 0)
                & (i_idx[None, :] + th[:, None] <= H - 1)).astype(F32)
    r0 = np.clip(cut_ox - 64, 0, H - 1)
    r1 = np.clip(cut_ox + 63, 0, H - 1)
    c0 = np.clip(cut_oy - 64, 0, W - 1)
    c1 = np.clip(cut_oy + 63, 0, W - 1)
    rm = ((i_idx[None, :] >= r0[:, None]) & (i_idx[None, :] <= r1[:, None])
          & cut[:, None]).astype(F32)
    cm = ((i_idx[None, :] >= c0[:, None]) & (i_idx[None, :] <= c1[:, None])
          & cut[:, None]).astype(F32)

    return {
        "xpad3": xpad3,
        "scl": scale,
        "bia": bias,
        "av": (-(rowvalid * rm)).astype(F32).reshape(B, 2, 128),
        "rv": rowvalid.reshape(B, 2, 128).copy(),
        "cm": cm,
        "lin": lin_off,
        "z": z_slot,
    }


# --------------------------------------------------------------------------
def _build_nc():
    # Wait-count discipline (this walrus flow allows only ONE sync wait per
    # TPB/DMA instruction):
    #  - all per-sample scalars/ints/cut-masks ship in ONE packed tensor
    #    (pars; ints bitcast into f32 columns), one DMA, one absorber copy
    #  - one 3D DMA per sample per direction (channels flattened into the
    #    row axis with constant 128-row subtile stride, order s = 2c+h)
    #  - all compute on DVE, where same-engine ordering needs no semaphores
    #  - pool bufs=S so there are no slot-reuse waits at all
    nc = bass.Bass(trn_type="TRN2")
    f32, i32 = mybir.dt.float32, mybir.dt.int32
    xpad3 = nc.dram_tensor("xpad3", [S, FLATR, TW], f32, kind="ExternalInput")
    pars = nc.dram_tensor("pars", [128, 8 * S + 128 * S], f32,
                          kind="ExternalInput")
    y = nc.dram_tensor("y", [S, C, H, W], f32, kind="ExternalOutput")

    with _SplitDrainTileContext(nc) as tc:
        with tc.tile_pool(name="const", bufs=1) as cpool, \
             tc.tile_pool(name="work", bufs=S) as wpool:
            parsT = cpool.tile([128, 8 * S + 128 * S], f32)
            scr = cpool.tile([128, 4], f32)
            nc.sync.dma_start(parsT, pars[:, :])
            # absorber: soak up the parsT DMA wait on DVE once, so the
            # 1-wait-budget TensorScalarPtr ops below never see it
            nc.vector.tensor_copy(scr[:, 0:1], parsT[:, 0:1])

            for b in range(S):
                T = wpool.tile([128, 6, 256], f32, tag="T")
                # OO[:,0] = affine image; OO[:,1] = masked output (also the
                # scratch bin for the reversed select of unflipped samples)
                OO = wpool.tile([128, 2, 6, 256], f32, tag="OO")
                Mh = wpool.tile([128, 2, 256], f32, tag="Mh")

                def iv(col, lo, hi, eng):
                    return nc.values_load(
                        parsT[0:1, col:col + 1].bitcast(i32),
                        engines=[eng], min_val=lo, max_val=hi,
                        skip_runtime_bounds_check=True)

                # ---- load: one 3D DMA, dynamic linear offset ----
                lin = iv(6 * S + 2 * b, 0, 32 * TW + 288, _ET.SP)
                src = AP(xpad3, b * (FLATR * TW) + lin,
                         [[TW, 128], [128 * TW, 6], [1, 256]])
                nc.sync.dma_start(T[:, :, :], src)

                # ---- rank-1 mask build (cutout rect + invalid rows) ----
                for h in (0, 1):
                    col = 2 * S + 2 * b + h
                    nc.vector.tensor_scalar(
                        Mh[:, h],
                        parsT[:, 8 * S + 128 * b:8 * S + 128 * b + 128]
                        .bitcast(mybir.dt.bfloat16),
                        parsT[:, col:col + 1],
                        parsT[:, 2 * S + col:2 * S + col + 1], _MULT, _ADD)

                # ---- affine selects (fwd -> slot0; reversed -> slot z,
                # which is slot0 for flipped samples, else the scratch
                # slot1 that the mask-apply overwrites in order) ----
                sc = parsT[:, b:b + 1]
                bi = parsT[:, S + b:S + b + 1]
                nc.vector.tensor_scalar(
                    OO[:, 0], T, sc, bi, _MULT, _ADD)
                z = iv(6 * S + 2 * b + 1, 0, 1, _ET.DVE)
                nc.vector.tensor_scalar(
                    OO[:, bass.ds(z, 1)],
                    T[:, :, ::-1].unsqueeze(1), sc, bi, _MULT, _ADD)

                # ---- mask apply (halves differ in per-partition scalars) ----
                for h in (0, 1):
                    nc.vector.tensor_mul(
                        OO[:, 1, h::2, :],
                        OO[:, 0, h::2, :],
                        Mh[:, h:h + 1, :].broadcast_to((128, 3, 256)))

                # ---- store: one 3D DMA into flat output rows ----
                dst = AP(y, b * (C * H * W),
                         [[256, 128], [128 * 256, 6], [1, 256]])
                nc.gpsimd.dma_start(dst, OO[:, 1])
    return nc


_NC = None


def _get_nc():
    global _NC
    if _NC is None:
        _NC = _build_nc()
    return _NC


def _shard(params, k):
    lo, hi = k * S, (k + 1) * S
    pars = np.zeros((128, 8 * S + 128 * S), np.float32)
    pars[:, 0:S] = params["scl"][lo:hi][None, :]
    pars[:, S:2 * S] = params["bia"][lo:hi][None, :]
    pars[:, 2 * S:4 * S] = params["av"][lo:hi].reshape(S * 2, 128).T
    pars[:, 4 * S:6 * S] = params["rv"][lo:hi].reshape(S * 2, 128).T
    ints = np.stack([params["lin"][lo:hi], params["z"][lo:hi]],
                    axis=1).reshape(2 * S).astype(np.int32)
    pars[:, 6 * S:8 * S] = ints.view(np.float32)[None, :]
    import ml_dtypes
    cmb = params["cm"][lo:hi].reshape(S * 256).astype(ml_dtypes.bfloat16)
    pars[:, 8 * S:] = np.broadcast_to(
        cmb.view(np.float32)[None, :], (128, S * 128))
    return {
        "xpad3": np.ascontiguousarray(params["xpad3"][lo:hi]),
        "pars": pars,
    }


def kernel(**inputs) -> np.ndarray:
    params = _derive_params(**{k: np.asarray(v) for k, v in inputs.items()})
    in_maps = [_shard(params, k) for k in range(N_CORES)]
    nc = _get_nc()
    res = run_bass_kernel_spmd(nc, in_maps, core_ids=list(range(N_CORES)))
    out = np.concatenate([np.asarray(r["y"], np.float32)
                          for r in res.results], axis=0)
    return np.ascontiguousarray(out)


if __name__ == "__main__":
    rng = np.random.default_rng(0)
    demo = {
        "x": rng.standard_normal((B, C, H, W)).astype(np.float32),
        "p": np.full((1,), 0.6, np.float32),
        "flip_u": rng.random(B).astype(np.float32),
        "bright_n": rng.standard_normal((B, 1, 1, 1)).astype(np.float32),
        "bright_u": rng.random((B, 1, 1, 1)).astype(np.float32),
        "contrast_n": rng.standard_normal((B, 1, 1, 1)).astype(np.float32),
        "contrast_u": rng.random((B, 1, 1, 1)).astype(np.float32),
        "trans_h": rng.integers(-16, 17, (B, 1, 1)).astype(np.int32),
        "trans_w": rng.integers(-16, 17, (B, 1, 1)).astype(np.int32),
        "trans_u": rng.random(B).astype(np.float32),
        "cut_ox": rng.integers(0, 257, (B, 1, 1)).astype(np.int32),
        "cut_oy": rng.integers(0, 257, (B, 1, 1)).astype(np.int32),
        "cut_u": rng.random(B).astype(np.float32),
    }
    out = kernel(**demo)
    print("kernel output:", out.shape, out.dtype)


# revision 37
# speedup vs baseline: 1.0665x; 1.0317x over previous
"""DiffAugment (flip / brightness / contrast / translation / cutout) on
Trainium2, data-parallel over 8 NeuronCores (8 samples per core).

Every per-sample augmentation decision is folded on the host into a small
set of per-sample parameters; the device runs one uniform SPMD Bass/Tile
program whose only data-dependent behavior is two runtime register values
per sample (a window offset and a flip-slot index).

Host builds xpad3 [S, 800, 544]: a flat row space (16 + 3*256 + 16; the
three channels tiled every 256 rows so one 3D DMA with constant 128-row
subtile stride covers all six (channel, row-half) subtiles; rows that a
translated window reads outside a channel's payload are zeroed later by
the rowvalid mask, so the 16-row end margins only provide address safety).
  cols [0:256]   raw x columns
  cols [256:544] translation ring B[k] = x[(k-16) % 255], with ring cells
                 B[16]/B[271] patched to x[.,255] for flipped+translated
                 samples (the one column where flip-then-translate and
                 translate-then-flip disagree).

Device, per sample (subtile s = 2c + h, image rows on partitions):
  T [128, 6, 256] <- ONE 3D DMA at dynamic lin_off = (16+th)*544 + col_off
  OO [128, 2, 6, 256] (DVE tensor_scalar affines, scale/bias per sample):
    slot0      = scale*T + bias
    slot ds(z) = scale*reverse(T) + bias   (z = 0 if flipped else 1, so a
                 flipped sample's reversed image overwrites slot0; slot1
                 is a scratch bin that the mask-apply overwrites in order)
  Mh[h] = cm * a[h] + rv[h]   (rank-1 mask realizing both the cutout
                               rectangle and out-of-range translation rows)
  OO[:,1,h::2,:] = OO[:,0,h::2,:] * Mh[h];  ONE 3D DMA stores OO[:,1].

This walrus flow packs at most ONE sync wait into any TPB/DMA instruction,
which dictates the structure: one DMA per sample per direction, all scalar
parameters (ints and the bf16 cutout masks bitcast into f32 columns) in a
single packed tensor with a one-time absorber copy, all compute on one
engine (DVE) where ordering is implicit, bufs=S so pool slots are never
reused, and a custom kernel-tail drain that takes its 17 semaphore waits
one NOP at a time.
"""
import sys
import numpy as np

for _p in ("/opt/trn_rl_repo",):
    if _p not in sys.path:
        sys.path.insert(0, _p)

import concourse.bass as bass
import concourse.mybir as mybir
from concourse.ap import AP
from concourse.tile import TileContext
from concourse.vector_clock import ScopedClock, VectorClock
from concourse.bass_utils import run_bass_kernel_spmd


class _SplitDrainTileContext(TileContext):
    """TileContext whose kernel-tail drain pre-absorbs its semaphore waits
    into one NOP per outstanding semaphore (this walrus flow packs at most
    one sync wait into any TPB instruction)."""

    def _drain_and_barrier(self, tick_clock, wait_clock):
        full = tick_clock.global_clock
        vals = [full[i] for i in range(27)]
        nz = [i for i, v in enumerate(vals) if v > 0]
        for i in nz:
            cv = [vals[j] if j == i else 0 for j in range(27)]
            nop = self.nc.sync.nop(nofuse=True)
            wait_clock.add_sem_waits(nop.ins,
                                     ScopedClock({None: VectorClock(cv)}))
        # the NOPs above already waited on every outstanding semaphore, so
        # the drain itself carries no sem waits (original code attaches all
        # of them to this one instruction, which overflows its wait slots)
        self.nc.sync.drain()
        self.nc.all_engine_barrier()
        assert self.sems is not None
        popped = self.nc._tile_sem_poison_stack.pop()
        assert popped is self._sem_poison
        self.nc.clear_and_free_semaphores(list(self.sems.allocated().values()))

N_CORES = 8
S = 8                      # samples per core
B, C, H, W = 64, 3, 256, 256
PAD_TOP = 16
FLATR = 800                # flat rows: 16 + 3*256 + 16
TW = 544                   # xpad2 col width: raw 256 + ring 288
NI = 4                     # ints per sample
F32 = np.float32

_IDENT = mybir.ActivationFunctionType.Identity
_ET = mybir.EngineType
_MULT = mybir.AluOpType.mult
_ADD = mybir.AluOpType.add


# --------------------------------------------------------------------------
# Host-side parameter derivation
# --------------------------------------------------------------------------
def _derive_params(x, p, flip_u, bright_n, bright_u, contrast_n, contrast_u,
                   trans_h, trans_w, trans_u, cut_ox, cut_oy, cut_u):
    x = np.asarray(x, np.float32)
    p = F32(np.asarray(p).reshape(()))
    flip_u = np.asarray(flip_u, np.float32).reshape(B)
    bright_n = np.asarray(bright_n, np.float32).reshape(B)
    bright_u = np.asarray(bright_u, np.float32).reshape(B)
    contrast_n = np.asarray(contrast_n, np.float32).reshape(B)
    contrast_u = np.asarray(contrast_u, np.float32).reshape(B)
    trans_h = np.asarray(trans_h).reshape(B).astype(np.int64)
    trans_w = np.asarray(trans_w).reshape(B).astype(np.int64)
    trans_u = np.asarray(trans_u, np.float32).reshape(B)
    cut_ox = np.asarray(cut_ox).reshape(B).astype(np.int64)
    cut_oy = np.asarray(cut_oy).reshape(B).astype(np.int64)
    cut_u = np.asarray(cut_u, np.float32).reshape(B)

    flip = flip_u < F32(0.5) * p
    trans = trans_u < p
    cut = cut_u < p

    th = np.where(trans, trans_h, 0)
    tw = np.where(trans, trans_w, 0)

    scale = np.where(contrast_u < p, np.exp2(contrast_n * F32(0.5)),
                     F32(1.0)).astype(F32)
    add = np.where(bright_u < p, bright_n * F32(0.2), F32(0.0)).astype(F32)
    bias = (add * scale).astype(F32)

    xflat = x.reshape(B, C * H, W)
    xpad3 = np.zeros((B, FLATR, TW), np.float32)
    xpad3[:, PAD_TOP:PAD_TOP + C * H, 0:256] = xflat
    ring_idx = np.concatenate([np.arange(239, 255),
                               np.arange(0, 255),
                               np.arange(0, 17)])
    xpad3[:, PAD_TOP:PAD_TOP + C * H, 256:544] = xflat[:, :, ring_idx]
    patched = flip & trans
    xpad3[patched, PAD_TOP:PAD_TOP + C * H, 256 + 16] = xflat[patched, :, 255]
    xpad3[patched, PAD_TOP:PAD_TOP + C * H, 256 + 271] = xflat[patched, :, 255]

    col_off = np.where(trans,
                       np.where(flip, 256 + 16 - tw, 256 + 16 + tw),
                       0).astype(np.int64)
    lin_off = ((PAD_TOP + th) * TW + col_off).astype(np.int32)
    z_slot = np.where(flip, 0, 1).astype(np.int32)

    i_idx = np.arange(H)
    rowvalid = ((i_idx[None, :] + th[:, None] >= 0)
                & (i_idx[None, :] + th[:, None] <= H - 1)).astype(F32)
    r0 = np.clip(cut_ox - 64, 0, H - 1)
    r1 = np.clip(cut_ox + 63, 0, H - 1)
    c0 = np.clip(cut_oy - 64, 0, W - 1)
    c1 = np.clip(cut_oy + 63, 0, W - 1)
    rm = ((i_idx[None, :] >= r0[:, None]) & (i_idx[None, :] <= r1[:, None])
          & cut[:, None]).astype(F32)
    cm = ((i_idx[None, :] >= c0[:, None]) & (i_idx[None, :] <= c1[:, None])
          & cut[:, None]).astype(F32)

    return {
        "xpad3": xpad3,
        "scl": scale,
        "bia": bias,
        "av": (-(rowvalid * rm)).astype(F32).reshape(B, 2, 128),
        "rv": rowvalid.reshape(B, 2, 128).copy(),
        "cm": cm,
        "lin": lin_off,
        "z": z_slot,
    }


# --------------------------------------------------------------------------
def _build_nc():
    # Wait-count discipline (this walrus flow allows only ONE sync wait per
    # TPB/DMA instruction):
    #  - all per-sample scalars/ints/cut-masks ship in ONE packed tensor
    #    (pars; ints bitcast into f32 columns), one DMA, one absorber copy
    #  - one 3D DMA per sample per direction (channels flattened into the
    #    row axis with constant 128-row subtile stride, order s = 2c+h)
    #  - all compute on DVE, where same-engine ordering needs no semaphores
    #  - pool bufs=S so there are no slot-reuse waits at all
    nc = bass.Bass(trn_type="TRN2")
    f32, i32 = mybir.dt.float32, mybir.dt.int32
    xpad3 = nc.dram_tensor("xpad3", [S, FLATR, TW], f32, kind="ExternalInput")
    parh = nc.dram_tensor("parh", [128, 8 * S], f32, kind="ExternalInput")
    cmb = nc.dram_tensor("cmb", [128, 128 * S], f32, kind="ExternalInput")
    y = nc.dram_tensor("y", [S, C, H, W], f32, kind="ExternalOutput")

    with _SplitDrainTileContext(nc) as tc:
        with tc.tile_pool(name="const", bufs=1) as cpool, \
             tc.tile_pool(name="work", bufs=S) as wpool:
            parsT = cpool.tile([128, 8 * S], f32)
            cmbT = cpool.tile([128, 128 * S], f32)
            scr = cpool.tile([128, 4], f32)
            # tiny head first so the per-sample reg-loads (and with them the
            # window-load DMAs) stop gating on the big cm block
            nc.sync.dma_start(parsT, parh[:, :])
            nc.sync.dma_start(cmbT, cmb[:, :])
            # absorbers: soak up both param-DMA waits on DVE once, so the
            # 1-wait-budget TensorScalarPtr ops below never see them
            nc.vector.tensor_copy(scr[:, 0:1], parsT[:, 0:1])
            nc.vector.tensor_copy(scr[:, 1:2], cmbT[:, 0:1])

            for b in range(S):
                T = wpool.tile([128, 6, 256], f32, tag="T")
                # OO[:,0] = affine image; OO[:,1] = masked output (also the
                # scratch bin for the reversed select of unflipped samples)
                OO = wpool.tile([128, 2, 6, 256], f32, tag="OO")
                Mh = wpool.tile([128, 2, 256], f32, tag="Mh")

                def iv(col, lo, hi, eng):
                    return nc.values_load(
                        parsT[0:1, col:col + 1].bitcast(i32),
                        engines=[eng], min_val=lo, max_val=hi,
                        skip_runtime_bounds_check=True)

                # ---- load: one 3D DMA, dynamic linear offset ----
                lin = iv(6 * S + 2 * b, 0, 32 * TW + 288, _ET.SP)
                src = AP(xpad3, b * (FLATR * TW) + lin,
                         [[TW, 128], [128 * TW, 6], [1, 256]])
                nc.sync.dma_start(T[:, :, :], src)

                # ---- rank-1 mask build (cutout rect + invalid rows) ----
                for h in (0, 1):
                    col = 2 * S + 2 * b + h
                    nc.vector.tensor_scalar(
                        Mh[:, h],
                        cmbT[:, 128 * b:128 * b + 128]
                        .bitcast(mybir.dt.bfloat16),
                        parsT[:, col:col + 1],
                        parsT[:, 2 * S + col:2 * S + col + 1], _MULT, _ADD)

                # ---- affine selects (fwd -> slot0; reversed -> slot z,
                # which is slot0 for flipped samples, else the scratch
                # slot1 that the mask-apply overwrites in order) ----
                sc = parsT[:, b:b + 1]
                bi = parsT[:, S + b:S + b + 1]
                nc.vector.tensor_scalar(
                    OO[:, 0], T, sc, bi, _MULT, _ADD)
                z = iv(6 * S + 2 * b + 1, 0, 1, _ET.DVE)
                nc.vector.tensor_scalar(
                    OO[:, bass.ds(z, 1)],
                    T[:, :, ::-1].unsqueeze(1), sc, bi, _MULT, _ADD)

                # ---- mask apply (halves differ in per-partition scalars) ----
                for h in (0, 1):
                    nc.vector.tensor_mul(
                        OO[:, 1, h::2, :],
                        OO[:, 0, h::2, :],
                        Mh[:, h:h + 1, :].broadcast_to((128, 3, 256)))

                # ---- store: one 3D DMA into flat output rows ----
                dst = AP(y, b * (C * H * W),
                         [[256, 128], [128 * 256, 6], [1, 256]])
                nc.gpsimd.dma_start(dst, OO[:, 1])
    return nc


_NC = None


def _get_nc():
    global _NC
    if _NC is None:
        _NC = _build_nc()
    return _NC


def _shard(params, k):
    lo, hi = k * S, (k + 1) * S
    pars = np.zeros((128, 8 * S), np.float32)
    pars[:, 0:S] = params["scl"][lo:hi][None, :]
    pars[:, S:2 * S] = params["bia"][lo:hi][None, :]
    pars[:, 2 * S:4 * S] = params["av"][lo:hi].reshape(S * 2, 128).T
    pars[:, 4 * S:6 * S] = params["rv"][lo:hi].reshape(S * 2, 128).T
    ints = np.stack([params["lin"][lo:hi], params["z"][lo:hi]],
                    axis=1).reshape(2 * S).astype(np.int32)
    pars[:, 6 * S:8 * S] = ints.view(np.float32)[None, :]
    import ml_dtypes
    cmb = params["cm"][lo:hi].reshape(S * 256).astype(ml_dtypes.bfloat16)
    cm_block = np.ascontiguousarray(
        np.broadcast_to(cmb.view(np.float32)[None, :], (128, S * 128)))
    return {
        "xpad3": np.ascontiguousarray(params["xpad3"][lo:hi]),
        "parh": pars,
        "cmb": cm_block,
    }


def kernel(**inputs) -> np.ndarray:
    params = _derive_params(**{k: np.asarray(v) for k, v in inputs.items()})
    in_maps = [_shard(params, k) for k in range(N_CORES)]
    nc = _get_nc()
    res = run_bass_kernel_spmd(nc, in_maps, core_ids=list(range(N_CORES)))
    out = np.concatenate([np.asarray(r["y"], np.float32)
                          for r in res.results], axis=0)
    return np.ascontiguousarray(out)


if __name__ == "__main__":
    rng = np.random.default_rng(0)
    demo = {
        "x": rng.standard_normal((B, C, H, W)).astype(np.float32),
        "p": np.full((1,), 0.6, np.float32),
        "flip_u": rng.random(B).astype(np.float32),
        "bright_n": rng.standard_normal((B, 1, 1, 1)).astype(np.float32),
        "bright_u": rng.random((B, 1, 1, 1)).astype(np.float32),
        "contrast_n": rng.standard_normal((B, 1, 1, 1)).astype(np.float32),
        "contrast_u": rng.random((B, 1, 1, 1)).astype(np.float32),
        "trans_h": rng.integers(-16, 17, (B, 1, 1)).astype(np.int32),
        "trans_w": rng.integers(-16, 17, (B, 1, 1)).astype(np.int32),
        "trans_u": rng.random(B).astype(np.float32),
        "cut_ox": rng.integers(0, 257, (B, 1, 1)).astype(np.int32),
        "cut_oy": rng.integers(0, 257, (B, 1, 1)).astype(np.int32),
        "cut_u": rng.random(B).astype(np.float32),
    }
    out = kernel(**demo)
    print("kernel output:", out.shape, out.dtype)
